# revision 5
# baseline (speedup 1.0000x reference)
"""AttentionPooling (segment_reduce) Trainium2 kernel.

att = sigmoid([input_rep, final_rep] @ W_lin.T + b_lin)
g   = att * (final_rep @ W_last.T + b_last)
out = segment_sum(g, graph_index, 16384)          # graph_index sorted

Strategy (8 NeuronCores, pure data-parallel, no collectives):
  graph_index is sorted, so a contiguous node range covers a contiguous
  graph range.  Host greedily packs whole graphs into "windows" of
  <= WIN_NODES nodes spanning <= 128 graphs; ~136 windows cover all 500k
  nodes = 8 cores x 17 windows.  Each core gets its windows as a padded
  node stream in feature-major bf16 layout (host pre-transposes + casts,
  so the device only does line-rate DMA and matmuls).  Per 128-node
  subtile the device does:
    ones[1,128].T @ biascat[1,512]      -> initializes PSUM with biases
                                           (K=1 matmuls row-packed via
                                           tile_position, 2 concurrent)
    xT_in.T   @ WlinT[:128]             -> att_pre cols   (accumulate)
    xT_fin0.T @ [WlinT[128:256]|WlastT[:128]]   (accumulate)
    xT_fin1.T @ [WlinT[256:]  |WlastT[128:]]    (accumulate)
    ACT: att = sigmoid(psum att cols)   -> bf16   (batched over 2 subtiles)
    DVE: g = att * psum val cols        -> bf16   (batched over 2 subtiles)
    DVE: onehot[n, j] = (iota[j] == local_graph_idx[n])      (bf16)
    PE : onehot.T @ g  += seg_psum[128 graphs, 256]   (whole window)
  After each window the [128, 256] f32 graph block is written out; host
  reassembles the window blocks into [16384, 256].
"""

import numpy as np
import ml_dtypes

import concourse.bass as bass
import concourse.bacc as bacc
import concourse.tile as tile
from concourse import mybir
from concourse import bass_utils
from concourse._compat import with_exitstack

P = 128
HID = 256
WIN_SUB = 29                     # subtiles (128 nodes) per window
WIN_NODES = WIN_SUB * P          # 3712
WINDOWS_PER_CORE = 17
N_CORES = 8
NUM_GRAPHS = 16384
GMAX = P                         # graph span per window

BF16 = mybir.dt.bfloat16
F32 = mybir.dt.float32
npbf16 = ml_dtypes.bfloat16


# ----------------------------------------------------------------------------
# host-side planning
# ----------------------------------------------------------------------------

def _build_windows(gi: np.ndarray, num_graphs: int):
    """Greedy windows: contiguous whole-graph ranges, graph span <= GMAX,
    node count <= WIN_NODES.  Returns list of (gbase, gcnt, nstart, ncnt)."""
    counts = np.bincount(gi, minlength=num_graphs)
    starts = np.concatenate([[0], np.cumsum(counts)])
    wins = []
    g = 0
    while g < num_graphs:
        base = g
        nodes = 0
        cnt = 0
        while g < num_graphs and cnt < GMAX and nodes + counts[g] <= WIN_NODES:
            nodes += int(counts[g])
            cnt += 1
            g += 1
        if cnt == 0:
            raise ValueError(f"graph {g} has {counts[g]} nodes > {WIN_NODES}")
        wins.append((base, cnt, int(starts[base]), nodes))
    return wins


# ----------------------------------------------------------------------------
# device kernel
# ----------------------------------------------------------------------------

@with_exitstack
def _device_kernel(ctx, tc, out_ap, ins, n_windows):
    nc = tc.nc
    xin_ap, xf0_ap, xf1_ap, idx_ap, wlin0_ap, wcat0_ap, wcat1_ap, \
        biascat_ap, ones_ap, iota_ap = ins

    consts = ctx.enter_context(tc.tile_pool(name="consts", bufs=1))
    xpool = ctx.enter_context(tc.tile_pool(name="x", bufs=2))
    apool = ctx.enter_context(tc.tile_pool(name="act", bufs=3))
    ohpool = ctx.enter_context(tc.tile_pool(name="oh", bufs=4))
    outpool = ctx.enter_context(tc.tile_pool(name="out", bufs=2))
    ps_sub = ctx.enter_context(tc.tile_pool(name="ps_sub", bufs=3, space="PSUM"))
    ps_seg = ctx.enter_context(tc.tile_pool(name="ps_seg", bufs=2, space="PSUM"))

    # constants, loaded once
    wlin0 = consts.tile([P, HID], BF16)
    nc.sync.dma_start(wlin0[:], wlin0_ap[:])
    wcat0 = consts.tile([P, 2 * HID], BF16)
    nc.sync.dma_start(wcat0[:], wcat0_ap[:])
    wcat1 = consts.tile([P, 2 * HID], BF16)
    nc.sync.dma_start(wcat1[:], wcat1_ap[:])
    biascat = consts.tile([P, 2 * HID], BF16)
    nc.sync.dma_start(biascat[:], biascat_ap[:])
    ones_t = consts.tile([P, P], BF16)
    nc.sync.dma_start(ones_t[:], ones_ap[:])
    iota_t = consts.tile([P, P], F32)
    nc.sync.dma_start(iota_t[:], iota_ap[:])
    idx_t = consts.tile([P, n_windows * WIN_SUB], F32)
    nc.sync.dma_start(idx_t[:], idx_ap[:])

    def emit_mms(ps, half, xin_t, xf0_t, xf1_t, col):
        """The 3 accumulating matmuls for one subtile into psum half."""
        o = 2 * HID * half
        nc.tensor.matmul(ps[:, o:o + HID], lhsT=xin_t[:, col:col + P],
                         rhs=wlin0[:, :], start=False, stop=False)
        nc.tensor.matmul(ps[:, o:o + 2 * HID], lhsT=xf0_t[:, col:col + P],
                         rhs=wcat0[:, :], start=False, stop=False)
        nc.tensor.matmul(ps[:, o:o + 2 * HID], lhsT=xf1_t[:, col:col + P],
                         rhs=wcat1[:, :], start=False, stop=True)

    def emit_bias(ps, half):
        o = 2 * HID * half
        nc.tensor.matmul(ps[:, o:o + 2 * HID],
                         lhsT=ones_t[32 * half:32 * half + 1, 0:P],
                         rhs=biascat[32 * half:32 * half + 1, :],
                         start=True, stop=False,
                         tile_position=(32 * half, 0))

    for w in range(n_windows):
        base = w * WIN_NODES
        xin_t = xpool.tile([P, WIN_NODES], BF16, tag="xin")
        nc.sync.dma_start(xin_t[:], xin_ap[:, base:base + WIN_NODES])
        xf0_t = xpool.tile([P, WIN_NODES], BF16, tag="xf0")
        nc.sync.dma_start(xf0_t[:], xf0_ap[:, base:base + WIN_NODES])
        xf1_t = xpool.tile([P, WIN_NODES], BF16, tag="xf1")
        nc.sync.dma_start(xf1_t[:], xf1_ap[:, base:base + WIN_NODES])

        seg = ps_seg.tile([P, HID], F32)

        n_pairs = WIN_SUB // 2            # 14 pairs + 1 odd subtile
        for pr in range(n_pairs + 1):
            odd = pr == n_pairs
            subs = [2 * pr] if odd else [2 * pr, 2 * pr + 1]
            ps = ps_sub.tile([P, 4 * HID], F32)   # 2 banks (one per subtile)
            for h, s in enumerate(subs):
                emit_bias(ps, h)
            for h, s in enumerate(subs):
                emit_mms(ps, h, xin_t, xf0_t, xf1_t, s * P)

            nsub = len(subs)
            # batched sigmoid over the att columns of both halves
            att = apool.tile([P, nsub * HID], BF16, tag="att")
            ps3 = ps[:, :].rearrange("p (two c) -> p two c", two=2)
            att3 = att[:, :].rearrange("p (two c) -> p two c", two=nsub) \
                if not odd else att[:, None, :]
            nc.scalar.activation(att3[:, 0:nsub, :], ps3[:, 0:nsub, 0:HID],
                                 mybir.ActivationFunctionType.Sigmoid)
            g_sb = apool.tile([P, nsub * HID], BF16, tag="g")
            g3 = g_sb[:, :].rearrange("p (two c) -> p two c", two=nsub) \
                if not odd else g_sb[:, None, :]
            nc.vector.tensor_tensor(g3[:, 0:nsub, :], att3[:, 0:nsub, :],
                                    ps3[:, 0:nsub, HID:2 * HID],
                                    op=mybir.AluOpType.mult)
            for h, s in enumerate(subs):
                t = w * WIN_SUB + s
                oh = ohpool.tile([P, P], BF16)
                nc.vector.tensor_scalar(oh[:], iota_t[:], idx_t[:, t:t + 1],
                                        None, op0=mybir.AluOpType.is_equal)
                nc.tensor.matmul(seg[:, :], lhsT=oh[:],
                                 rhs=g_sb[:, h * HID:(h + 1) * HID],
                                 start=(s == 0), stop=(s == WIN_SUB - 1))

        out_t = outpool.tile([P, HID], F32)
        nc.scalar.copy(out_t[:], seg[:, :])
        nc.sync.dma_start(out_ap[w * P:(w + 1) * P, :], out_t[:])


def build_module(n_windows=WINDOWS_PER_CORE):
    nc = bacc.Bacc("TRN2", debug=False, num_devices=N_CORES)
    nn = n_windows * WIN_NODES
    ins = [
        nc.dram_tensor("xin", [P, nn], BF16, kind="ExternalInput").ap(),
        nc.dram_tensor("xf0", [P, nn], BF16, kind="ExternalInput").ap(),
        nc.dram_tensor("xf1", [P, nn], BF16, kind="ExternalInput").ap(),
        nc.dram_tensor("idx", [P, nn // P], F32, kind="ExternalInput").ap(),
        nc.dram_tensor("wlin0", [P, HID], BF16, kind="ExternalInput").ap(),
        nc.dram_tensor("wcat0", [P, 2 * HID], BF16, kind="ExternalInput").ap(),
        nc.dram_tensor("wcat1", [P, 2 * HID], BF16, kind="ExternalInput").ap(),
        nc.dram_tensor("biascat", [P, 2 * HID], BF16, kind="ExternalInput").ap(),
        nc.dram_tensor("ones", [P, P], BF16, kind="ExternalInput").ap(),
        nc.dram_tensor("iota", [P, P], F32, kind="ExternalInput").ap(),
    ]
    out_ap = nc.dram_tensor("out", [n_windows * P, HID], F32,
                            kind="ExternalOutput").ap()
    with tile.TileContext(nc) as tc:
        _device_kernel(tc, out_ap, ins, n_windows)
    nc.compile()
    return nc


# ----------------------------------------------------------------------------
# host-side data prep
# ----------------------------------------------------------------------------

def _prep(inputs, n_windows):
    gi = np.asarray(inputs["graph_index"]).astype(np.int64)
    x_in = np.asarray(inputs["input_rep"], dtype=np.float32)
    x_fin = np.asarray(inputs["final_rep"], dtype=np.float32)
    W_lin = np.asarray(inputs["W_lin"], dtype=np.float32)
    b_lin = np.asarray(inputs["b_lin"], dtype=np.float32)
    W_last = np.asarray(inputs["W_last"], dtype=np.float32)
    b_last = np.asarray(inputs["b_last"], dtype=np.float32)

    if np.any(np.diff(gi) < 0):
        order = np.argsort(gi, kind="stable")
        gi = gi[order]
        x_in = x_in[order]
        x_fin = x_fin[order]

    wins = _build_windows(gi, NUM_GRAPHS)
    budget = N_CORES * n_windows
    assert len(wins) <= budget, f"{len(wins)} windows > budget {budget}"
    wins = wins + [(NUM_GRAPHS, 0, len(gi), 0)] * (budget - len(wins))

    x_in_b = x_in.astype(npbf16)
    x_fin_b = x_fin.astype(npbf16)

    WlinT = W_lin.T.astype(npbf16)    # [384, 256]
    WlastT = W_last.T.astype(npbf16)  # [256, 256]
    wlin0 = np.ascontiguousarray(WlinT[0:P])
    wcat0 = np.ascontiguousarray(
        np.concatenate([WlinT[P:2 * P], WlastT[0:P]], axis=1))
    wcat1 = np.ascontiguousarray(
        np.concatenate([WlinT[2 * P:3 * P], WlastT[P:2 * P]], axis=1))
    biascat = np.tile(np.concatenate([b_lin, b_last])[None, :],
                      (P, 1)).astype(npbf16)
    ones_t = np.ones((P, P), npbf16)
    iota_t = np.tile(np.arange(P, dtype=np.float32)[None, :], (P, 1))

    nn = n_windows * WIN_NODES
    in_maps = []
    for c in range(N_CORES):
        xin_p = np.zeros((P, nn), npbf16)
        xf0_p = np.zeros((P, nn), npbf16)
        xf1_p = np.zeros((P, nn), npbf16)
        idx_p = np.full((P, nn // P), -1.0, np.float32)
        for j in range(n_windows):
            gb, gc, ns, ncnt = wins[c * n_windows + j]
            if ncnt == 0:
                continue
            off = j * WIN_NODES
            xin_p[:, off:off + ncnt] = x_in_b[ns:ns + ncnt].T
            xf0_p[:, off:off + ncnt] = x_fin_b[ns:ns + ncnt, 0:P].T
            xf1_p[:, off:off + ncnt] = x_fin_b[ns:ns + ncnt, P:2 * P].T
            flat = np.full((WIN_NODES,), -1.0, np.float32)
            flat[0:ncnt] = (gi[ns:ns + ncnt] - gb).astype(np.float32)
            cols = slice(off // P, (off + WIN_NODES) // P)
            idx_p[:, cols] = flat.reshape(-1, P).T
        in_maps.append({
            "xin": xin_p, "xf0": xf0_p, "xf1": xf1_p, "idx": idx_p,
            "wlin0": wlin0, "wcat0": wcat0, "wcat1": wcat1,
            "biascat": biascat, "ones": ones_t, "iota": iota_t,
        })
    return wins, in_maps


def _assemble(wins, results, n_windows):
    out = np.zeros((NUM_GRAPHS, HID), np.float32)
    for c in range(N_CORES):
        res = results[c]["out"]
        for j in range(n_windows):
            gb, gc, _, _ = wins[c * n_windows + j]
            if gc == 0:
                continue
            out[gb:gb + gc] = res[j * P:j * P + gc]
    return out


# ----------------------------------------------------------------------------
# entry point
# ----------------------------------------------------------------------------

_CACHE = {}
LAST_RESULTS = None


def kernel(**inputs) -> np.ndarray:
    global LAST_RESULTS
    gi = np.asarray(inputs["graph_index"]).astype(np.int64)
    n_wins_needed = len(_build_windows(np.sort(gi), NUM_GRAPHS))
    n_windows = max(WINDOWS_PER_CORE, -(-n_wins_needed // N_CORES))
    if n_windows not in _CACHE:
        _CACHE[n_windows] = build_module(n_windows)
    nc = _CACHE[n_windows]
    wins, in_maps = _prep(inputs, n_windows)
    res = bass_utils.run_bass_kernel_spmd(
        nc, in_maps, core_ids=list(range(N_CORES)))
    LAST_RESULTS = res
    return _assemble(wins, res.results, n_windows)
